# revision 87
# baseline (speedup 1.0000x reference)
"""Multi-head self-attention (RoPE, causal) Trainium2 Bass kernel.

Sharding: head-parallel across 8 NeuronCores. Core c owns heads {2c, 2c+1}
for both batch rows. Each core computes its heads' QKV projection, RoPE,
causal flash attention (scores kept transposed [k, q]), softmax
normalization, and a partial output projection against its 128 columns of
W_o. The host sums the 8 partial projections (the "all-reduce").

v3: contiguous DMA layouts, j-outer QKV interleaved with the first scores
group, 2-way row-tiled scores (h0/h1 on PE tiles (0,0)/(64,0)), 2-way
col-tiled AV (M=64, heads stacked in one PSUM bank), 4-way col-tiled
denominator matmuls software-pipelined against the AV pairs (hides the
same-bank WAW drain), full-tile reciprocal + single-copy DENRB (den banks
memset to 1.0 so garbage rows stay finite), rope swap DMAs spread over
three engine DMA rings (each ring is depth-1 paced), fine-grained HAM
warmup sized to the input-DMA latency, priority-ordered input DMAs,
triple-buffered scores PSUM during QKV, hand-scheduled consumer-first
post-QKV interleave, per-2-et streamed output DMA.

Self-contained: hardcodes B=2, S=2048, D=1024, H=16, d_k=64.
"""
import numpy as np
import ml_dtypes

B, S, D, H, DK = 2, 2048, 1024, 16, 64
NCORES = 8
THETA = 10000.0
BS = B * S                   # 4096 flattened tokens (b-major)
KT = D // 128                # 8 contraction tiles
P = 128

bf16 = ml_dtypes.bfloat16

_CACHED_NC = None


def _host_prep(x, token_positions, W_qkv, W_o):
    """Build per-core DRAM input dicts (numpy, bf16), contiguous layouts."""
    cast = lambda a: np.ascontiguousarray(a).astype(bf16)
    X2 = np.asarray(x, np.float32).reshape(BS, D)
    # xt[p, j, kt, s] = X2[512j+s, 128kt+p]  -> per-partition contiguous 8KB
    xt = cast(X2.T.reshape(KT, P, 8, 512).transpose(1, 2, 0, 3))

    pos = np.asarray(token_positions, np.float64)
    inv = THETA ** (-np.arange(0, DK, 2, dtype=np.float64) / DK)   # [32]
    ang = pos[:, None] * inv[None, :]                              # [S, 32]
    cosv = np.cos(ang).T.astype(np.float32)                        # [32, S]
    sinv = np.sin(ang).T.astype(np.float32)
    COS = cast(np.tile(cosv, (4, 1)))                              # [128, S]
    SINS = cast(np.concatenate([-sinv, sinv, -sinv, sinv], 0))     # [128, S]

    perm = np.concatenate([np.arange(0, 64, 2), np.arange(1, 64, 2)])
    tri = cast(np.triu(np.ones((P, P), np.float32)))               # [k,q]: q>=k

    # sel[u, jm, m] = 1 iff u == 32*(2*jm + m//64): broadcasts recip rows
    sel = np.zeros((P, 2, P), np.float32)
    for jm in range(2):
        for m in range(P):
            sel[32 * (2 * jm + m // 64), jm, m] = 1.0

    def warrange(Wrows):                   # [128 rows, D] -> [p, kt, m]
        return cast(Wrows.T.reshape(KT, P, P).transpose(1, 0, 2))

    Wqkv = np.asarray(W_qkv, np.float32)
    Wo = np.asarray(W_o, np.float32)
    maps = []
    for c in range(NCORES):
        hA = 2 * c
        rows = np.concatenate([(hA + 0) * 64 + perm, (hA + 1) * 64 + perm])
        rows_v = np.concatenate([(hA + 0) * 64 + np.arange(64),
                                 (hA + 1) * 64 + np.arange(64)])
        maps.append({
            "xt": xt,
            "wq": warrange(Wqkv[rows]),
            "wk": warrange(Wqkv[D + rows]),
            "wv": warrange(Wqkv[2 * D + rows_v]),
            "wo": cast(Wo[:, P * c:P * c + P].T),                  # [128, 1024]
            "cos": COS,
            "sin": SINS,
            "tri": tri,
            "sel": cast(sel),
        })
    return maps


def _build_nc(debug=False):
    """Trace + compile the per-core Bass module (same program on all cores)."""
    from contextlib import ExitStack
    import concourse.bacc as bacc
    import concourse.mybir as mybir
    import concourse.tile as tile
    from concourse.bass import ts

    f32 = mybir.dt.float32
    bf = mybir.dt.bfloat16
    EXP = mybir.ActivationFunctionType.Exp

    nc = bacc.Bacc("TRN2", target_bir_lowering=False, debug=False,
                   enable_asserts=False)

    xt_d = nc.dram_tensor("xt", [P, 8, KT, 512], bf, kind="ExternalInput").ap()
    wq_d = nc.dram_tensor("wq", [P, KT, P], bf, kind="ExternalInput").ap()
    wk_d = nc.dram_tensor("wk", [P, KT, P], bf, kind="ExternalInput").ap()
    wv_d = nc.dram_tensor("wv", [P, KT, P], bf, kind="ExternalInput").ap()
    wo_d = nc.dram_tensor("wo", [P, D], bf, kind="ExternalInput").ap()
    cos_d = nc.dram_tensor("cos", [P, S], bf, kind="ExternalInput").ap()
    sin_d = nc.dram_tensor("sin", [P, S], bf, kind="ExternalInput").ap()
    tri_d = nc.dram_tensor("tri", [P, P], bf, kind="ExternalInput").ap()
    sel_d = nc.dram_tensor("sel", [P, 2, P], bf, kind="ExternalInput").ap()
    yt_d = nc.dram_tensor("yt", [8, P, 8, 512], bf, kind="ExternalOutput").ap()

    with tile.TileContext(nc) as tc, ExitStack() as ctx:
        # ---- kernel-lifetime pools ----
        pp = ctx.enter_context(tc.tile_pool(name="persist", bufs=1))
        WO = pp.tile([P, D], bf, tag="wo")
        TRI = pp.tile([P, P], bf, tag="tri")
        SEL2 = pp.tile([P, 2, P], bf, tag="sel")
        ONES = pp.tile([P, 1], bf, tag="ones")
        WARM = pp.tile([P, 512], bf, tag="warm")
        QA = pp.tile([P, BS], bf, tag="qa")
        KA = pp.tile([P, BS], bf, tag="ka")
        VT = pp.tile([P, BS], bf, tag="vt")
        OACC = [pp.tile([P, 4, 512], f32, tag=f"oacc{b}", name=f"oacc{b}")
                for b in range(B)]
        OPR = [pp.tile([P, 512], bf, tag=f"opr{jj}", name=f"opr{jj}")
               for jj in range(8)]
        # bf16 broadcast-ready reciprocal rows at partitions {0,32,64,96}
        DENRB = [pp.tile([P, 512], bf, tag=f"denrb{q4}", name=f"denrb{q4}")
                 for q4 in range(4)]
        drp = ctx.enter_context(tc.tile_pool(name="denr", bufs=2))
        vbp = ctx.enter_context(tc.tile_pool(name="vbig", bufs=4))
        ybp = ctx.enter_context(tc.tile_pool(name="ybig", bufs=2))
        ptb = ctx.enter_context(tc.tile_pool(name="ptbig", bufs=16))
        swp = ctx.enter_context(tc.tile_pool(name="swap", bufs=2))
        COS = pp.tile([P, S], bf, tag="cos")
        SIN = pp.tile([P, S], bf, tag="sin")

        # ---- PE warmup (HAM) + ACT exp-table preload, before any DMA dep ----
        nc.gpsimd.memset(WARM[:], 0.0)
        nc.gpsimd.memset(ONES[:], 1.0)
        for q4 in range(4):
            nc.gpsimd.memset(DENRB[q4][:], 0.0)
        scratch = pp.tile([1, 8], bf, tag="scratch")
        nc.scalar.activation(scratch[:], WARM[0:1, 0:8], EXP, scale=1.0)

        pts = {}
        vba = {}
        scps = None  # created after the QKV block (PSUM budget)

        def transposes(b):
            VBA = vbp.tile([P, 16, 64], bf, tag="vb", name=f"vba{b}")
            VBB = vbp.tile([P, 16, 64], bf, tag="vb", name=f"vbb{b}")
            nc.sync.dma_start_transpose(VBA[:], VT[0:64, b * S:(b + 1) * S])
            nc.sync.dma_start_transpose(VBB[:], VT[64:128, b * S:(b + 1) * S])
            vba[b] = (VBA, VBB)

        rope_sw = {}

        def rope_swaps(b):
            """Issue the partition-block swap DMAs early, spread across
            engine DMA queues: each queue is depth-1 paced (~43 GB/s), so
            putting all 8x128KB on one ring serializes to ~20us."""
            bsl = ts(b, S)
            sws = []
            qs = [[nc.gpsimd, nc.sync, nc.gpsimd, nc.sync],
                  [nc.scalar, nc.gpsimd, nc.sync, nc.scalar]]
            for ai, A in enumerate((QA, KA)):
                SW = swp.tile([P, S], bf, tag="sw", name="sw")
                for blkk in range(4):  # partition-block swap 0<->1, 2<->3
                    src = blkk ^ 1
                    qs[ai][blkk].dma_start(SW[32 * blkk:32 * blkk + 32, :],
                                           A[32 * src:32 * src + 32, bsl])
                sws.append(SW)
            rope_sw[b] = sws

        def rope_muls(b):
            for A, SW in zip((QA, KA), rope_sw[b]):
                for ch in range(2):   # 1024-wide chunks over this batch
                    asl = slice(b * S + 1024 * ch,
                                b * S + 1024 * ch + 1024)
                    csl = ts(ch, 1024)
                    nc.vector.tensor_mul(A[:, asl], A[:, asl], COS[:, csl])
                    nc.vector.tensor_mul(SW[:, csl], SW[:, csl],
                                         SIN[:, csl])
                    nc.vector.tensor_add(A[:, asl], A[:, asl],
                                         SW[:, csl])

        def rope_cswaps(b, ch):
            """Chunked (1024-col) swap issue, multi-queue (b0 streaming)."""
            asl = slice(b * S + 1024 * ch, b * S + 1024 * ch + 1024)
            qs = [[nc.gpsimd, nc.sync, nc.gpsimd, nc.sync],
                  [nc.scalar, nc.gpsimd, nc.sync, nc.scalar]]
            sws = []
            for ai, A in enumerate((QA, KA)):
                SW = swp.tile([P, 1024], bf, tag="swc", name="swc")
                for blkk in range(4):
                    src = blkk ^ 1
                    qs[ai][blkk].dma_start(SW[32 * blkk:32 * blkk + 32, :],
                                           A[32 * src:32 * src + 32, asl])
                sws.append(SW)
            rope_sw[(b, ch)] = sws

        def rope_cmuls(b, ch):
            csl = ts(ch, 1024)
            asl = slice(b * S + 1024 * ch, b * S + 1024 * ch + 1024)
            for A, SW in zip((QA, KA), rope_sw[(b, ch)]):
                nc.vector.tensor_mul(A[:, asl], A[:, asl], COS[:, csl])
                nc.vector.tensor_mul(SW[:], SW[:], SIN[:, csl])
                nc.vector.tensor_add(A[:, asl], A[:, asl], SW[:])

        def scores_slab(b, ilist, pool, width, psp, off_range=None):
            """Row-tiled (h0/h1 interleaved) score matmuls + exp for i in
            ilist. pt columns are global-q minus qs_i. off_range optionally
            restricts the q-window (chunk-split emission)."""
            for i in ilist:
                qs_i = 512 * (i // 4)
                qext = S - qs_i
                blk = b * S + 128 * i
                if (b, i, 0) not in pts:
                    for h in range(2):
                        pts[(b, i, h)] = pool.tile([P, width], bf, tag="pt",
                                                   name=f"pt{b}_{i}_{h}")
                lo, hi = (0, qext) if off_range is None else \
                    (off_range[0], min(off_range[1], qext))
                off = lo
                while off < hi:
                    w = min(1024, hi - off)
                    ps2 = [psp.tile([P, 1024], f32, tag="sc", name=f"sc{h}")
                           for h in range(2)]
                    vf = max(0, 128 * i - (qs_i + off))
                    for qc in range(0, w, 512):
                        sub = min(512, w - qc)
                        q0 = qs_i + off + qc
                        if q0 + sub <= 128 * i:
                            continue  # fully masked chunk
                        # skip causal-masked leading cols (exp reads [vf:w],
                        # so the unwritten psum region is never consumed)
                        mvf = max(0, 128 * i - q0)
                        for h in range(2):
                            hsl = slice(64 * h, 64 * h + 64)
                            nc.tensor.matmul(
                                ps2[h][:, qc + mvf:qc + sub],
                                lhsT=KA[hsl, blk:blk + 128],
                                rhs=QA[hsl, b * S + q0 + mvf:
                                        b * S + q0 + sub],
                                start=True, stop=True)
                    for h in range(2):
                        pt = pts[(b, i, h)]
                        if vf < w:
                            nc.scalar.activation(pt[:, off + vf:off + w],
                                                 ps2[h][:, vf:w], EXP,
                                                 scale=0.125)
                        if vf > 0:
                            nc.gpsimd.memset(pt[:, off:off + vf], 0.0)
                    off += w
                dc = 128 * i - qs_i
                if lo <= dc < hi:  # diagonal mask only in its owning chunk
                    for h in range(2):
                        nc.gpsimd.tensor_mul(pts[(b, i, h)][:, dc:dc + 128],
                                             pts[(b, i, h)][:, dc:dc + 128],
                                             TRI[:])

        # ---- QKV projection, j-outer, interleaved with b0/g0 scores ----
        with tc.tile_pool(name="xtp", bufs=3) as xtp, \
             tc.tile_pool(name="wp", bufs=1) as wp, \
             tc.tile_pool(name="scpsb", bufs=3, space="PSUM") as scps_big, \
             tc.tile_pool(name="qkvps", bufs=2, space="PSUM") as qps:
            WQ = wp.tile([P, KT, P], bf, tag="wq")
            WK = wp.tile([P, KT, P], bf, tag="wk")
            WV = wp.tile([P, KT, P], bf, tag="wv")
            # issue order = DMA priority: first-needed tensors first
            nc.sync.dma_start(WQ[:], wq_d)
            nc.sync.dma_start(WK[:], wk_d)
            nc.sync.dma_start(WV[:], wv_d)

            # HAM warmup: fine-grained garbage matmuls while wq/xt0 stream in
            for wi in range(56):
                pw = qps.tile([P, 512], f32, tag="qkv", name="pw")
                nc.tensor.matmul(pw[:, 0:128], lhsT=WARM[:, 0:128],
                                 rhs=WARM[:, 0:128], start=True, stop=True)

            xts = {}

            def fetch(j, split=1):
                t = xtp.tile([P, KT, 512], bf, tag="xt", name=f"xt{j}")
                for s in range(split):
                    k0, k1 = KT * s // split, KT * (s + 1) // split
                    nc.sync.dma_start(t[:, k0:k1], xt_d[:, j, k0:k1])
                xts[j] = t

            fetch(0, split=4)   # j0 kt-split: first MMs start at 1/4 arrival
            fetch(1)
            nc.sync.dma_start(COS[:], cos_d)
            nc.sync.dma_start(SIN[:], sin_d)
            nc.sync.dma_start(WO[:], wo_d)
            nc.sync.dma_start(TRI[:], tri_d)
            nc.sync.dma_start(SEL2[:], sel_d)
            ncpy = 0
            for j in range(8):
                if j + 2 < 8:
                    fetch(j + 2)
                for W, DST in ((WQ, QA), (WK, KA), (WV, VT)):
                    ps = qps.tile([P, 512], f32, tag="qkv", name="ps")
                    for kt in range(KT):
                        nc.tensor.matmul(ps[:], lhsT=W[:, kt, :],
                                         rhs=xts[j][:, kt, :],
                                         start=(kt == 0), stop=(kt == KT - 1))
                    # early j: scalar (exp hasn't started, DVE owns rope);
                    # late j: vector
                    if j < 6:
                        nc.scalar.copy(DST[:, ts(j, 512)], ps[:])
                    else:
                        nc.vector.tensor_copy(DST[:, ts(j, 512)], ps[:])
                    ncpy += 1
                del xts[j]
                # b0 rope streams chunked: swaps at j=1/3 (right after the
                # last needed scalar copy), muls at j=2/4 while DVE is idle
                if j == 1:
                    rope_cswaps(0, 0)
                elif j == 2:
                    rope_cmuls(0, 0)
                elif j == 3:
                    rope_cswaps(0, 1)
                    transposes(0)
                elif j == 4:
                    rope_cmuls(0, 1)
                if j == 7:       # b1 rope swaps go out as soon as QA/KA done
                    rope_swaps(1)
                # one-j delay so slab MMs never head-of-line block the next
                # QKV group while waiting on rope(0)'s DVE muls
                if j >= 5:
                    scores_slab(0, [2 * (j - 5), 2 * (j - 5) + 1], ptb, 2048,
                                scps_big)
                if j == 7:   # finish b0/g0 in-block: ACT stays fed across
                    scores_slab(0, [6, 7], ptb, 2048, scps_big)

        scps = ctx.enter_context(tc.tile_pool(name="scps", bufs=2,
                                              space="PSUM"))

        # ---- attention consumers + remaining scores groups ----
        pts_small = ctx.enter_context(tc.tile_pool(name="ptsm", bufs=16))
        pay = ctx.enter_context(tc.tile_pool(name="pay", bufs=2, space="PSUM"))
        dnp = ctx.enter_context(tc.tile_pool(name="dnp", bufs=2, space="PSUM"))

        den_banks = {}

        def ilist_for(g, j):
            return [i for i in range(8 * g, 8 * g + 8) if i <= 4 * j + 3]

        def av_den(b, g, j):
            """Col-tiled AV accumulation for query block j plus 4-way-tiled
            denominator matmuls, software-pipelined so den's same-bank WAW
            stall hides under the next i's AV pair; PSUM -> OACC copy/add."""
            ilist = ilist_for(g, j)
            if not ilist:
                return
            VBA, VBB = vba[b]
            pa = pay.tile([P, 512], f32, tag="pay", name="pa")
            half = j // 2
            key = (b, half)
            if key not in den_banks:
                den_banks[key] = dnp.tile([P, 512], f32, tag="den",
                                          name=f"den{b}_{half}")
                # keep unwritten rows finite so the full-tile reciprocal
                # in div_oproj can't produce inf/NaN for the SEL matmul
                nc.vector.memset(den_banks[key][:], 1.0)
            dbank = den_banks[key]
            glast = 0 if j <= 1 else 1

            def av_pair(n, i):
                qs_i = 512 * (i // 4)
                o0 = 512 * j - qs_i
                # leading cols with q < 128i are causal-masked zeros in pt:
                # skip them (identical accumulation, shorter stream)
                vfj = max(0, 128 * i - 512 * j)
                for h, VB in ((0, VBA), (1, VBB)):
                    nc.tensor.matmul(
                        pa[64 * h:64 * h + 64, vfj:512],
                        lhsT=VB[:, i, :],
                        rhs=pts[(b, i, h)][:, o0 + vfj:o0 + 512],
                        start=(n == 0), stop=(n == len(ilist) - 1),
                        skip_group_check=True)

            def den_pair(n, i):
                qs_i = 512 * (i // 4)
                o0 = 512 * j - qs_i
                vfj = max(0, 128 * i - 512 * j)
                for h in range(2):
                    r = 32 * (2 * (j % 2) + h)
                    nc.tensor.matmul(
                        dbank[r:r + 1, vfj:512],
                        lhsT=ONES[:, 0:1],
                        rhs=pts[(b, i, h)][:, o0 + vfj:o0 + 512],
                        start=(g == 0 and n == 0),
                        stop=(g == glast and n == len(ilist) - 1),
                        skip_group_check=True,
                        tile_position=(0, r))

            for n, i in enumerate(ilist):
                av_pair(n, i)
                if n >= 1:
                    den_pair(n - 1, ilist[n - 1])
            den_pair(len(ilist) - 1, ilist[-1])
            if g == 0:
                nc.vector.tensor_copy(OACC[b][:, j, :], pa[:])
            else:
                nc.vector.tensor_add(OACC[b][:, j, :], OACC[b][:, j, :],
                                     pa[:])

        def div_oproj(b, half, borrow=False):
            """Reciprocal of denominators, SEL broadcast, OPR, o_proj,
            store. borrow=True: park o_proj pairs in the (then-idle)
            scores PSUM pool for 3x-deep pipelining and 2-et copies --
            only safe once no scps generation is still pending."""
            q4 = b * 2 + half
            dbank = den_banks.pop((b, half))
            DENR = drp.tile([P, 512], f32, tag="denr", name="denr")
            nc.vector.reciprocal_approx_fast(DENR[:], dbank[:])
            nc.vector.tensor_copy(DENRB[q4][:], DENR[:])
            for j in (2 * half, 2 * half + 1):
                jj = b * 4 + j
                pb = pay.tile([P, 512], f32, tag="pay", name="pb")
                nc.tensor.matmul(pb[:], lhsT=SEL2[:, j % 2, :],
                                 rhs=DENRB[q4][:], start=True, stop=True)
                nc.vector.tensor_mul(OPR[jj][:], OACC[b][:, j, :], pb[:])
                yb = ybp.tile([P, 8, 512], bf, tag="yb", name="yb")
                py2 = None
                for et in range(8):
                    if borrow:
                        if et % 2 == 0:
                            py2 = scps.tile([P, 1024], f32, tag="sc",
                                            name="pyw")
                        py = py2[:, ts(et % 2, 512)]
                    else:
                        py = pay.tile([P, 512], f32, tag="pay",
                                      name="py")[:]
                    nc.tensor.matmul(py, lhsT=WO[:, ts(et, P)],
                                     rhs=OPR[jj][:], start=True, stop=True)
                    if borrow:
                        if et % 2 == 1:
                            if et % 4 == 1:
                                nc.vector.tensor_copy(
                                    yb[:, et - 1:et + 1, :], py2[:])
                            else:
                                nc.scalar.copy(
                                    yb[:, et - 1:et + 1, :], py2[:])
                            nc.sync.dma_start(
                                yt_d[jj][:, et - 1:et + 1, :],
                                yb[:, et - 1:et + 1, :])
                        continue
                    # b1 runs at the tail when ACT is exp-free: go 1:1 v/s
                    if (et % 2 == 1) if b == 1 else (et % 3 == 2):
                        nc.scalar.copy(yb[:, et, :], py)
                    else:
                        nc.vector.tensor_copy(yb[:, et, :], py)
                    if et % 2 == 1:  # stream out per 2-et chunk
                        nc.sync.dma_start(yt_d[jj][:, et - 1:et + 1, :],
                                          yb[:, et - 1:et + 1, :])

        # ---- hand-scheduled interleave ----
        # Consumers first at the QKV transition (no exp dependency, keeps the
        # PE hot while DVE does rope(1)/div chains); b1's wide slabs start as
        # early as rope(1) allows so ACT never starves.
        def sl(b, i0, pool, w):
            scores_slab(b, [i0, i0 + 1], pool, w, scps)

        transposes(1)
        av_den(0, 0, 0)
        sl(0, 8, pts_small, 1024)
        av_den(0, 0, 1)
        av_den(0, 0, 2)
        rope_muls(1)
        sl(0, 10, pts_small, 1024)
        av_den(0, 0, 3)
        div_oproj(0, 0)
        sl(0, 12, pts_small, 1024)
        sl(0, 14, pts_small, 1024)
        av_den(0, 1, 2)
        sl(1, 0, ptb, 2048)
        av_den(0, 1, 3)
        sl(1, 2, ptb, 2048)
        div_oproj(0, 1)
        sl(1, 4, ptb, 2048)
        av_den(1, 0, 0)
        sl(1, 6, ptb, 2048)
        av_den(1, 0, 1)
        av_den(1, 0, 2)
        sl(1, 8, pts_small, 1024)
        av_den(1, 0, 3)
        div_oproj(1, 0)
        sl(1, 10, pts_small, 1024)
        sl(1, 12, pts_small, 1024)
        av_den(1, 1, 2)
        sl(1, 14, pts_small, 1024)
        av_den(1, 1, 3)
        div_oproj(1, 1, borrow=True)

    nc.compile()
    return nc


def get_nc():
    global _CACHED_NC
    if _CACHED_NC is None:
        _CACHED_NC = _build_nc()
    return _CACHED_NC


def run_on_hw(in_maps, **kwargs):
    from concourse.bass_utils import run_bass_kernel_spmd
    nc = get_nc()
    return run_bass_kernel_spmd(nc, in_maps, core_ids=list(range(NCORES)),
                                **kwargs)


def gather(results):
    acc = np.zeros((D, BS), np.float32)
    for r in results:
        yt = np.asarray(r["yt"]).astype(np.float32)    # [8, 128, 8, 512]
        acc += yt.transpose(2, 1, 0, 3).reshape(D, BS)
    return np.ascontiguousarray(acc.T).reshape(B, S, D).astype(np.float32)


def kernel(x, token_positions, W_qkv, W_o):
    in_maps = _host_prep(x, token_positions, W_qkv, W_o)
    res = run_on_hw(in_maps)
    return gather(res.results)



# revision 88
# speedup vs baseline: 1.0115x; 1.0115x over previous
"""Multi-head self-attention (RoPE, causal) Trainium2 Bass kernel.

Sharding: head-parallel across 8 NeuronCores. Core c owns heads {2c, 2c+1}
for both batch rows. Each core computes its heads' QKV projection, RoPE,
causal flash attention (scores kept transposed [k, q]), softmax
normalization, and a partial output projection against its 128 columns of
W_o. The host sums the 8 partial projections (the "all-reduce").

v3: contiguous DMA layouts, j-outer QKV interleaved with the first scores
group, 2-way row-tiled scores (h0/h1 on PE tiles (0,0)/(64,0)), 2-way
col-tiled AV (M=64, heads stacked in one PSUM bank), 4-way col-tiled
denominator matmuls software-pipelined against the AV pairs (hides the
same-bank WAW drain), full-tile reciprocal + single-copy DENRB (den banks
memset to 1.0 so garbage rows stay finite), rope swap DMAs spread over
three engine DMA rings (each ring is depth-1 paced), fine-grained HAM
warmup sized to the input-DMA latency, priority-ordered input DMAs,
triple-buffered scores PSUM during QKV, hand-scheduled consumer-first
post-QKV interleave, per-2-et streamed output DMA.

Self-contained: hardcodes B=2, S=2048, D=1024, H=16, d_k=64.
"""
import numpy as np
import ml_dtypes

B, S, D, H, DK = 2, 2048, 1024, 16, 64
NCORES = 8
THETA = 10000.0
BS = B * S                   # 4096 flattened tokens (b-major)
KT = D // 128                # 8 contraction tiles
P = 128

bf16 = ml_dtypes.bfloat16

_CACHED_NC = None


def _host_prep(x, token_positions, W_qkv, W_o):
    """Build per-core DRAM input dicts (numpy, bf16), contiguous layouts."""
    cast = lambda a: np.ascontiguousarray(a).astype(bf16)
    X2 = np.asarray(x, np.float32).reshape(BS, D)
    # xt[p, j, kt, s] = X2[512j+s, 128kt+p]  -> per-partition contiguous 8KB
    xt = cast(X2.T.reshape(KT, P, 8, 512).transpose(1, 2, 0, 3))

    pos = np.asarray(token_positions, np.float64)
    inv = THETA ** (-np.arange(0, DK, 2, dtype=np.float64) / DK)   # [32]
    ang = pos[:, None] * inv[None, :]                              # [S, 32]
    cosv = np.cos(ang).T.astype(np.float32)                        # [32, S]
    sinv = np.sin(ang).T.astype(np.float32)
    COS = cast(np.tile(cosv, (4, 1)))                              # [128, S]
    SINS = cast(np.concatenate([-sinv, sinv, -sinv, sinv], 0))     # [128, S]

    perm = np.concatenate([np.arange(0, 64, 2), np.arange(1, 64, 2)])
    tri = cast(np.triu(np.ones((P, P), np.float32)))               # [k,q]: q>=k

    # sel[u, jm, m] = 1 iff u == 32*(2*jm + m//64): broadcasts recip rows
    sel = np.zeros((P, 2, P), np.float32)
    for jm in range(2):
        for m in range(P):
            sel[32 * (2 * jm + m // 64), jm, m] = 1.0

    def warrange(Wrows):                   # [128 rows, D] -> [p, kt, m]
        return cast(Wrows.T.reshape(KT, P, P).transpose(1, 0, 2))

    Wqkv = np.asarray(W_qkv, np.float32)
    Wo = np.asarray(W_o, np.float32)
    maps = []
    for c in range(NCORES):
        hA = 2 * c
        rows = np.concatenate([(hA + 0) * 64 + perm, (hA + 1) * 64 + perm])
        rows_v = np.concatenate([(hA + 0) * 64 + np.arange(64),
                                 (hA + 1) * 64 + np.arange(64)])
        maps.append({
            "xt": xt,
            "wq": warrange(Wqkv[rows]),
            "wk": warrange(Wqkv[D + rows]),
            "wv": warrange(Wqkv[2 * D + rows_v]),
            "wo": cast(Wo[:, P * c:P * c + P].T),                  # [128, 1024]
            "cos": COS,
            "sin": SINS,
            "tri": tri,
            "sel": cast(sel),
        })
    return maps


def _build_nc(debug=False):
    """Trace + compile the per-core Bass module (same program on all cores)."""
    from contextlib import ExitStack
    import concourse.bacc as bacc
    import concourse.mybir as mybir
    import concourse.tile as tile
    from concourse.bass import ts

    f32 = mybir.dt.float32
    bf = mybir.dt.bfloat16
    EXP = mybir.ActivationFunctionType.Exp

    nc = bacc.Bacc("TRN2", target_bir_lowering=False, debug=False,
                   enable_asserts=False)

    xt_d = nc.dram_tensor("xt", [P, 8, KT, 512], bf, kind="ExternalInput").ap()
    wq_d = nc.dram_tensor("wq", [P, KT, P], bf, kind="ExternalInput").ap()
    wk_d = nc.dram_tensor("wk", [P, KT, P], bf, kind="ExternalInput").ap()
    wv_d = nc.dram_tensor("wv", [P, KT, P], bf, kind="ExternalInput").ap()
    wo_d = nc.dram_tensor("wo", [P, D], bf, kind="ExternalInput").ap()
    cos_d = nc.dram_tensor("cos", [P, S], bf, kind="ExternalInput").ap()
    sin_d = nc.dram_tensor("sin", [P, S], bf, kind="ExternalInput").ap()
    tri_d = nc.dram_tensor("tri", [P, P], bf, kind="ExternalInput").ap()
    sel_d = nc.dram_tensor("sel", [P, 2, P], bf, kind="ExternalInput").ap()
    yt_d = nc.dram_tensor("yt", [8, P, 8, 512], bf, kind="ExternalOutput").ap()

    with tile.TileContext(nc) as tc, ExitStack() as ctx:
        # ---- kernel-lifetime pools ----
        pp = ctx.enter_context(tc.tile_pool(name="persist", bufs=1))
        WO = pp.tile([P, D], bf, tag="wo")
        TRI = pp.tile([P, P], bf, tag="tri")
        SEL2 = pp.tile([P, 2, P], bf, tag="sel")
        ONES = pp.tile([P, 1], bf, tag="ones")
        WARM = pp.tile([P, 512], bf, tag="warm")
        QA = pp.tile([P, BS], bf, tag="qa")
        KA = pp.tile([P, BS], bf, tag="ka")
        VT = pp.tile([P, BS], bf, tag="vt")
        OACC = [pp.tile([P, 4, 512], f32, tag=f"oacc{b}", name=f"oacc{b}")
                for b in range(B)]
        OPR = [pp.tile([P, 512], bf, tag=f"opr{jj}", name=f"opr{jj}")
               for jj in range(8)]
        # bf16 broadcast-ready reciprocal rows at partitions {0,32,64,96}
        DENRB = [pp.tile([P, 512], bf, tag=f"denrb{q4}", name=f"denrb{q4}")
                 for q4 in range(4)]
        drp = ctx.enter_context(tc.tile_pool(name="denr", bufs=2))
        vbp = ctx.enter_context(tc.tile_pool(name="vbig", bufs=4))
        ybp = ctx.enter_context(tc.tile_pool(name="ybig", bufs=2))
        ptb = ctx.enter_context(tc.tile_pool(name="ptbig", bufs=16))
        swp = ctx.enter_context(tc.tile_pool(name="swap", bufs=2))
        COS = pp.tile([P, S], bf, tag="cos")
        SIN = pp.tile([P, S], bf, tag="sin")

        # ---- PE warmup (HAM) + ACT exp-table preload, before any DMA dep ----
        nc.gpsimd.memset(WARM[:], 0.0)
        nc.gpsimd.memset(ONES[:], 1.0)
        for q4 in range(4):
            nc.gpsimd.memset(DENRB[q4][:], 0.0)
        scratch = pp.tile([1, 8], bf, tag="scratch")
        nc.scalar.activation(scratch[:], WARM[0:1, 0:8], EXP, scale=1.0)

        pts = {}
        vba = {}
        scps = None  # created after the QKV block (PSUM budget)

        def transposes(b):
            VBA = vbp.tile([P, 16, 64], bf, tag="vb", name=f"vba{b}")
            VBB = vbp.tile([P, 16, 64], bf, tag="vb", name=f"vbb{b}")
            nc.sync.dma_start_transpose(VBA[:], VT[0:64, b * S:(b + 1) * S])
            nc.sync.dma_start_transpose(VBB[:], VT[64:128, b * S:(b + 1) * S])
            vba[b] = (VBA, VBB)

        rope_sw = {}

        def rope_swaps(b):
            """Issue the partition-block swap DMAs early, spread across
            engine DMA queues: each queue is depth-1 paced (~43 GB/s), so
            putting all 8x128KB on one ring serializes to ~20us."""
            bsl = ts(b, S)
            sws = []
            qs = [[nc.gpsimd, nc.sync, nc.gpsimd, nc.sync],
                  [nc.scalar, nc.gpsimd, nc.sync, nc.scalar]]
            for ai, A in enumerate((QA, KA)):
                SW = swp.tile([P, S], bf, tag="sw", name="sw")
                for blkk in range(4):  # partition-block swap 0<->1, 2<->3
                    src = blkk ^ 1
                    qs[ai][blkk].dma_start(SW[32 * blkk:32 * blkk + 32, :],
                                           A[32 * src:32 * src + 32, bsl])
                sws.append(SW)
            rope_sw[b] = sws

        def rope_muls(b):
            for A, SW in zip((QA, KA), rope_sw[b]):
                for ch in range(2):   # 1024-wide chunks over this batch
                    asl = slice(b * S + 1024 * ch,
                                b * S + 1024 * ch + 1024)
                    csl = ts(ch, 1024)
                    nc.vector.tensor_mul(A[:, asl], A[:, asl], COS[:, csl])
                    nc.vector.tensor_mul(SW[:, csl], SW[:, csl],
                                         SIN[:, csl])
                    nc.vector.tensor_add(A[:, asl], A[:, asl],
                                         SW[:, csl])

        def rope_cswaps(b, ch):
            """Chunked (1024-col) swap issue, multi-queue (b0 streaming)."""
            asl = slice(b * S + 1024 * ch, b * S + 1024 * ch + 1024)
            qs = [[nc.gpsimd, nc.sync, nc.gpsimd, nc.sync],
                  [nc.scalar, nc.gpsimd, nc.sync, nc.scalar]]
            sws = []
            for ai, A in enumerate((QA, KA)):
                SW = swp.tile([P, 1024], bf, tag="swc", name="swc")
                for blkk in range(4):
                    src = blkk ^ 1
                    qs[ai][blkk].dma_start(SW[32 * blkk:32 * blkk + 32, :],
                                           A[32 * src:32 * src + 32, asl])
                sws.append(SW)
            rope_sw[(b, ch)] = sws

        def rope_cmuls(b, ch):
            csl = ts(ch, 1024)
            asl = slice(b * S + 1024 * ch, b * S + 1024 * ch + 1024)
            for A, SW in zip((QA, KA), rope_sw[(b, ch)]):
                nc.vector.tensor_mul(A[:, asl], A[:, asl], COS[:, csl])
                nc.vector.tensor_mul(SW[:], SW[:], SIN[:, csl])
                nc.vector.tensor_add(A[:, asl], A[:, asl], SW[:])

        def scores_slab(b, ilist, pool, width, psp, off_range=None):
            """Row-tiled (h0/h1 interleaved) score matmuls + exp for i in
            ilist. pt columns are global-q minus qs_i. off_range optionally
            restricts the q-window (chunk-split emission)."""
            for i in ilist:
                qs_i = 512 * (i // 4)
                qext = S - qs_i
                blk = b * S + 128 * i
                if (b, i, 0) not in pts:
                    for h in range(2):
                        pts[(b, i, h)] = pool.tile([P, width], bf, tag="pt",
                                                   name=f"pt{b}_{i}_{h}")
                lo, hi = (0, qext) if off_range is None else \
                    (off_range[0], min(off_range[1], qext))
                off = lo
                while off < hi:
                    w = min(1024, hi - off)
                    ps2 = [psp.tile([P, 1024], f32, tag="sc", name=f"sc{h}")
                           for h in range(2)]
                    vf = max(0, 128 * i - (qs_i + off))
                    for qc in range(0, w, 512):
                        sub = min(512, w - qc)
                        q0 = qs_i + off + qc
                        if q0 + sub <= 128 * i:
                            continue  # fully masked chunk
                        for h in range(2):
                            hsl = slice(64 * h, 64 * h + 64)
                            nc.tensor.matmul(
                                ps2[h][:, qc:qc + sub],
                                lhsT=KA[hsl, blk:blk + 128],
                                rhs=QA[hsl, b * S + q0:b * S + q0 + sub],
                                start=True, stop=True)
                    for h in range(2):
                        pt = pts[(b, i, h)]
                        if vf < w:
                            nc.scalar.activation(pt[:, off + vf:off + w],
                                                 ps2[h][:, vf:w], EXP,
                                                 scale=0.125)
                        if vf > 0:
                            nc.gpsimd.memset(pt[:, off:off + vf], 0.0)
                    off += w
                dc = 128 * i - qs_i
                if lo <= dc < hi:  # diagonal mask only in its owning chunk
                    for h in range(2):
                        nc.gpsimd.tensor_mul(pts[(b, i, h)][:, dc:dc + 128],
                                             pts[(b, i, h)][:, dc:dc + 128],
                                             TRI[:])

        # ---- QKV projection, j-outer, interleaved with b0/g0 scores ----
        with tc.tile_pool(name="xtp", bufs=3) as xtp, \
             tc.tile_pool(name="wp", bufs=1) as wp, \
             tc.tile_pool(name="scpsb", bufs=3, space="PSUM") as scps_big, \
             tc.tile_pool(name="qkvps", bufs=2, space="PSUM") as qps:
            WQ = wp.tile([P, KT, P], bf, tag="wq")
            WK = wp.tile([P, KT, P], bf, tag="wk")
            WV = wp.tile([P, KT, P], bf, tag="wv")
            # issue order = DMA priority: first-needed tensors first
            nc.sync.dma_start(WQ[:], wq_d)
            nc.sync.dma_start(WK[:], wk_d)
            nc.sync.dma_start(WV[:], wv_d)

            # HAM warmup: fine-grained garbage matmuls while wq/xt0 stream in
            for wi in range(56):
                pw = qps.tile([P, 512], f32, tag="qkv", name="pw")
                nc.tensor.matmul(pw[:, 0:128], lhsT=WARM[:, 0:128],
                                 rhs=WARM[:, 0:128], start=True, stop=True)

            xts = {}

            def fetch(j, split=1):
                t = xtp.tile([P, KT, 512], bf, tag="xt", name=f"xt{j}")
                for s in range(split):
                    k0, k1 = KT * s // split, KT * (s + 1) // split
                    nc.sync.dma_start(t[:, k0:k1], xt_d[:, j, k0:k1])
                xts[j] = t

            fetch(0, split=4)   # j0 kt-split: first MMs start at 1/4 arrival
            fetch(1)
            nc.sync.dma_start(COS[:], cos_d)
            nc.sync.dma_start(SIN[:], sin_d)
            nc.sync.dma_start(WO[:], wo_d)
            nc.sync.dma_start(TRI[:], tri_d)
            nc.sync.dma_start(SEL2[:], sel_d)
            ncpy = 0
            for j in range(8):
                if j + 2 < 8:
                    fetch(j + 2)
                for W, DST in ((WQ, QA), (WK, KA), (WV, VT)):
                    ps = qps.tile([P, 512], f32, tag="qkv", name="ps")
                    for kt in range(KT):
                        nc.tensor.matmul(ps[:], lhsT=W[:, kt, :],
                                         rhs=xts[j][:, kt, :],
                                         start=(kt == 0), stop=(kt == KT - 1))
                    # early j: scalar (exp hasn't started, DVE owns rope);
                    # late j: vector
                    if j < 6:
                        nc.scalar.copy(DST[:, ts(j, 512)], ps[:])
                    else:
                        nc.vector.tensor_copy(DST[:, ts(j, 512)], ps[:])
                    ncpy += 1
                del xts[j]
                # b0 rope streams chunked: swaps at j=1/3 (right after the
                # last needed scalar copy), muls at j=2/4 while DVE is idle
                if j == 1:
                    rope_cswaps(0, 0)
                elif j == 2:
                    rope_cmuls(0, 0)
                elif j == 3:
                    rope_cswaps(0, 1)
                    transposes(0)
                elif j == 4:
                    rope_cmuls(0, 1)
                if j == 7:       # b1 rope swaps go out as soon as QA/KA done
                    rope_swaps(1)
                # one-j delay so slab MMs never head-of-line block the next
                # QKV group while waiting on rope(0)'s DVE muls
                if j >= 5:
                    scores_slab(0, [2 * (j - 5), 2 * (j - 5) + 1], ptb, 2048,
                                scps_big)
                if j == 7:   # finish b0/g0 in-block: ACT stays fed across
                    scores_slab(0, [6, 7], ptb, 2048, scps_big)

        scps = ctx.enter_context(tc.tile_pool(name="scps", bufs=2,
                                              space="PSUM"))

        # ---- attention consumers + remaining scores groups ----
        pts_small = ctx.enter_context(tc.tile_pool(name="ptsm", bufs=16))
        pay = ctx.enter_context(tc.tile_pool(name="pay", bufs=2, space="PSUM"))
        dnp = ctx.enter_context(tc.tile_pool(name="dnp", bufs=2, space="PSUM"))

        den_banks = {}

        def ilist_for(g, j):
            return [i for i in range(8 * g, 8 * g + 8) if i <= 4 * j + 3]

        def av_den(b, g, j):
            """Col-tiled AV accumulation for query block j plus 4-way-tiled
            denominator matmuls, software-pipelined so den's same-bank WAW
            stall hides under the next i's AV pair; PSUM -> OACC copy/add."""
            ilist = ilist_for(g, j)
            if not ilist:
                return
            VBA, VBB = vba[b]
            pa = pay.tile([P, 512], f32, tag="pay", name="pa")
            half = j // 2
            key = (b, half)
            if key not in den_banks:
                den_banks[key] = dnp.tile([P, 512], f32, tag="den",
                                          name=f"den{b}_{half}")
                # keep unwritten rows finite so the full-tile reciprocal
                # in div_oproj can't produce inf/NaN for the SEL matmul
                nc.vector.memset(den_banks[key][:], 1.0)
            dbank = den_banks[key]
            glast = 0 if j <= 1 else 1

            def av_pair(n, i):
                qs_i = 512 * (i // 4)
                o0 = 512 * j - qs_i
                # leading cols with q < 128i are causal-masked zeros in pt:
                # skip them (identical accumulation, shorter stream)
                vfj = max(0, 128 * i - 512 * j)
                for h, VB in ((0, VBA), (1, VBB)):
                    nc.tensor.matmul(
                        pa[64 * h:64 * h + 64, vfj:512],
                        lhsT=VB[:, i, :],
                        rhs=pts[(b, i, h)][:, o0 + vfj:o0 + 512],
                        start=(n == 0), stop=(n == len(ilist) - 1),
                        skip_group_check=True)

            def den_pair(n, i):
                qs_i = 512 * (i // 4)
                o0 = 512 * j - qs_i
                vfj = max(0, 128 * i - 512 * j)
                for h in range(2):
                    r = 32 * (2 * (j % 2) + h)
                    nc.tensor.matmul(
                        dbank[r:r + 1, vfj:512],
                        lhsT=ONES[:, 0:1],
                        rhs=pts[(b, i, h)][:, o0 + vfj:o0 + 512],
                        start=(g == 0 and n == 0),
                        stop=(g == glast and n == len(ilist) - 1),
                        skip_group_check=True,
                        tile_position=(0, r))

            for n, i in enumerate(ilist):
                av_pair(n, i)
                if n >= 1:
                    den_pair(n - 1, ilist[n - 1])
            den_pair(len(ilist) - 1, ilist[-1])
            if g == 0:
                nc.vector.tensor_copy(OACC[b][:, j, :], pa[:])
            else:
                nc.vector.tensor_add(OACC[b][:, j, :], OACC[b][:, j, :],
                                     pa[:])

        def div_oproj(b, half, borrow=False):
            """Reciprocal of denominators, SEL broadcast, OPR, o_proj,
            store. borrow=True: park o_proj pairs in the (then-idle)
            scores PSUM pool for 3x-deep pipelining and 2-et copies --
            only safe once no scps generation is still pending."""
            q4 = b * 2 + half
            dbank = den_banks.pop((b, half))
            DENR = drp.tile([P, 512], f32, tag="denr", name="denr")
            nc.vector.reciprocal_approx_fast(DENR[:], dbank[:])
            nc.vector.tensor_copy(DENRB[q4][:], DENR[:])
            for j in (2 * half, 2 * half + 1):
                jj = b * 4 + j
                pb = pay.tile([P, 512], f32, tag="pay", name="pb")
                nc.tensor.matmul(pb[:], lhsT=SEL2[:, j % 2, :],
                                 rhs=DENRB[q4][:], start=True, stop=True)
                nc.vector.tensor_mul(OPR[jj][:], OACC[b][:, j, :], pb[:])
                yb = ybp.tile([P, 8, 512], bf, tag="yb", name="yb")
                py2 = None
                for et in range(8):
                    if borrow:
                        if et % 2 == 0:
                            py2 = scps.tile([P, 1024], f32, tag="sc",
                                            name="pyw")
                        py = py2[:, ts(et % 2, 512)]
                    else:
                        py = pay.tile([P, 512], f32, tag="pay",
                                      name="py")[:]
                    nc.tensor.matmul(py, lhsT=WO[:, ts(et, P)],
                                     rhs=OPR[jj][:], start=True, stop=True)
                    if borrow:
                        if et % 2 == 1:
                            if et % 4 == 1:
                                nc.vector.tensor_copy(
                                    yb[:, et - 1:et + 1, :], py2[:])
                            else:
                                nc.scalar.copy(
                                    yb[:, et - 1:et + 1, :], py2[:])
                            nc.sync.dma_start(
                                yt_d[jj][:, et - 1:et + 1, :],
                                yb[:, et - 1:et + 1, :])
                        continue
                    # b1 runs at the tail when ACT is exp-free: go 1:1 v/s
                    if (et % 2 == 1) if b == 1 else (et % 3 == 2):
                        nc.scalar.copy(yb[:, et, :], py)
                    else:
                        nc.vector.tensor_copy(yb[:, et, :], py)
                    if et % 2 == 1:  # stream out per 2-et chunk
                        nc.sync.dma_start(yt_d[jj][:, et - 1:et + 1, :],
                                          yb[:, et - 1:et + 1, :])

        # ---- hand-scheduled interleave ----
        # Consumers first at the QKV transition (no exp dependency, keeps the
        # PE hot while DVE does rope(1)/div chains); b1's wide slabs start as
        # early as rope(1) allows so ACT never starves.
        def sl(b, i0, pool, w):
            scores_slab(b, [i0, i0 + 1], pool, w, scps)

        transposes(1)
        av_den(0, 0, 0)
        sl(0, 8, pts_small, 1024)
        av_den(0, 0, 1)
        av_den(0, 0, 2)
        rope_muls(1)
        sl(0, 10, pts_small, 1024)
        av_den(0, 0, 3)
        div_oproj(0, 0)
        sl(0, 12, pts_small, 1024)
        sl(0, 14, pts_small, 1024)
        av_den(0, 1, 2)
        sl(1, 0, ptb, 2048)
        av_den(0, 1, 3)
        sl(1, 2, ptb, 2048)
        div_oproj(0, 1)
        sl(1, 4, ptb, 2048)
        av_den(1, 0, 0)
        sl(1, 6, ptb, 2048)
        av_den(1, 0, 1)
        av_den(1, 0, 2)
        sl(1, 8, pts_small, 1024)
        av_den(1, 0, 3)
        div_oproj(1, 0)
        sl(1, 10, pts_small, 1024)
        sl(1, 12, pts_small, 1024)
        av_den(1, 1, 2)
        sl(1, 14, pts_small, 1024)
        av_den(1, 1, 3)
        div_oproj(1, 1, borrow=True)

    nc.compile()
    return nc


def get_nc():
    global _CACHED_NC
    if _CACHED_NC is None:
        _CACHED_NC = _build_nc()
    return _CACHED_NC


def run_on_hw(in_maps, **kwargs):
    from concourse.bass_utils import run_bass_kernel_spmd
    nc = get_nc()
    return run_bass_kernel_spmd(nc, in_maps, core_ids=list(range(NCORES)),
                                **kwargs)


def gather(results):
    acc = np.zeros((D, BS), np.float32)
    for r in results:
        yt = np.asarray(r["yt"]).astype(np.float32)    # [8, 128, 8, 512]
        acc += yt.transpose(2, 1, 0, 3).reshape(D, BS)
    return np.ascontiguousarray(acc.T).reshape(B, S, D).astype(np.float32)


def kernel(x, token_positions, W_qkv, W_o):
    in_maps = _host_prep(x, token_positions, W_qkv, W_o)
    res = run_on_hw(in_maps)
    return gather(res.results)



# revision 90
# speedup vs baseline: 1.0323x; 1.0206x over previous
"""Multi-head self-attention (RoPE, causal) Trainium2 Bass kernel.

Sharding: head-parallel across 8 NeuronCores. Core c owns heads {2c, 2c+1}
for both batch rows. Each core computes its heads' QKV projection, RoPE,
causal flash attention (scores kept transposed [k, q]), softmax
normalization, and a partial output projection against its 128 columns of
W_o. The host sums the 8 partial projections (the "all-reduce").

v3: contiguous DMA layouts, j-outer QKV interleaved with the first scores
group, 2-way row-tiled scores (h0/h1 on PE tiles (0,0)/(64,0)), 2-way
col-tiled AV (M=64, heads stacked in one PSUM bank), 4-way col-tiled
denominator matmuls software-pipelined against the AV pairs (hides the
same-bank WAW drain), full-tile reciprocal + single-copy DENRB (den banks
memset to 1.0 so garbage rows stay finite), rope swap DMAs spread over
three engine DMA rings (each ring is depth-1 paced), fine-grained HAM
warmup sized to the input-DMA latency, priority-ordered input DMAs,
triple-buffered scores PSUM during QKV, hand-scheduled consumer-first
post-QKV interleave, per-2-et streamed output DMA.

Self-contained: hardcodes B=2, S=2048, D=1024, H=16, d_k=64.
"""
import numpy as np
import ml_dtypes

B, S, D, H, DK = 2, 2048, 1024, 16, 64
NCORES = 8
THETA = 10000.0
BS = B * S                   # 4096 flattened tokens (b-major)
KT = D // 128                # 8 contraction tiles
P = 128

bf16 = ml_dtypes.bfloat16

_CACHED_NC = None


def _host_prep(x, token_positions, W_qkv, W_o):
    """Build per-core DRAM input dicts (numpy, bf16), contiguous layouts."""
    cast = lambda a: np.ascontiguousarray(a).astype(bf16)
    X2 = np.asarray(x, np.float32).reshape(BS, D)
    # xt[p, j, kt, s] = X2[512j+s, 128kt+p]  -> per-partition contiguous 8KB
    xt = cast(X2.T.reshape(KT, P, 8, 512).transpose(1, 2, 0, 3))

    pos = np.asarray(token_positions, np.float64)
    inv = THETA ** (-np.arange(0, DK, 2, dtype=np.float64) / DK)   # [32]
    ang = pos[:, None] * inv[None, :]                              # [S, 32]
    cosv = np.cos(ang).T.astype(np.float32)                        # [32, S]
    sinv = np.sin(ang).T.astype(np.float32)
    COS = cast(np.tile(cosv, (4, 1)))                              # [128, S]
    SINS = cast(np.concatenate([-sinv, sinv, -sinv, sinv], 0))     # [128, S]

    perm = np.concatenate([np.arange(0, 64, 2), np.arange(1, 64, 2)])
    tri = cast(np.triu(np.ones((P, P), np.float32)))               # [k,q]: q>=k

    # sel[u, jm, m] = 1 iff u == 32*(2*jm + m//64): broadcasts recip rows
    sel = np.zeros((P, 2, P), np.float32)
    for jm in range(2):
        for m in range(P):
            sel[32 * (2 * jm + m // 64), jm, m] = 1.0

    def warrange(Wrows):                   # [128 rows, D] -> [p, kt, m]
        return cast(Wrows.T.reshape(KT, P, P).transpose(1, 0, 2))

    Wqkv = np.asarray(W_qkv, np.float32)
    Wo = np.asarray(W_o, np.float32)
    maps = []
    for c in range(NCORES):
        hA = 2 * c
        rows = np.concatenate([(hA + 0) * 64 + perm, (hA + 1) * 64 + perm])
        rows_v = np.concatenate([(hA + 0) * 64 + np.arange(64),
                                 (hA + 1) * 64 + np.arange(64)])
        maps.append({
            "xt": xt,
            "wq": warrange(Wqkv[rows]),
            "wk": warrange(Wqkv[D + rows]),
            "wv": warrange(Wqkv[2 * D + rows_v]),
            "wo": cast(Wo[:, P * c:P * c + P].T),                  # [128, 1024]
            "cos": COS,
            "sin": SINS,
            "tri": tri,
            "sel": cast(sel),
        })
    return maps


def _build_nc(debug=False):
    """Trace + compile the per-core Bass module (same program on all cores)."""
    from contextlib import ExitStack
    import concourse.bacc as bacc
    import concourse.mybir as mybir
    import concourse.tile as tile
    from concourse.bass import ts

    f32 = mybir.dt.float32
    bf = mybir.dt.bfloat16
    EXP = mybir.ActivationFunctionType.Exp

    nc = bacc.Bacc("TRN2", target_bir_lowering=False, debug=False,
                   enable_asserts=False)

    xt_d = nc.dram_tensor("xt", [P, 8, KT, 512], bf, kind="ExternalInput").ap()
    wq_d = nc.dram_tensor("wq", [P, KT, P], bf, kind="ExternalInput").ap()
    wk_d = nc.dram_tensor("wk", [P, KT, P], bf, kind="ExternalInput").ap()
    wv_d = nc.dram_tensor("wv", [P, KT, P], bf, kind="ExternalInput").ap()
    wo_d = nc.dram_tensor("wo", [P, D], bf, kind="ExternalInput").ap()
    cos_d = nc.dram_tensor("cos", [P, S], bf, kind="ExternalInput").ap()
    sin_d = nc.dram_tensor("sin", [P, S], bf, kind="ExternalInput").ap()
    tri_d = nc.dram_tensor("tri", [P, P], bf, kind="ExternalInput").ap()
    sel_d = nc.dram_tensor("sel", [P, 2, P], bf, kind="ExternalInput").ap()
    yt_d = nc.dram_tensor("yt", [8, P, 8, 512], bf, kind="ExternalOutput").ap()

    with tile.TileContext(nc) as tc, ExitStack() as ctx:
        # ---- kernel-lifetime pools ----
        pp = ctx.enter_context(tc.tile_pool(name="persist", bufs=1))
        WO = pp.tile([P, D], bf, tag="wo")
        TRI = pp.tile([P, P], bf, tag="tri")
        SEL2 = pp.tile([P, 2, P], bf, tag="sel")
        ONES = pp.tile([P, 1], bf, tag="ones")
        WARM = pp.tile([P, 512], bf, tag="warm")
        QA = pp.tile([P, BS], bf, tag="qa")
        KA = pp.tile([P, BS], bf, tag="ka")
        VT = pp.tile([P, BS], bf, tag="vt")
        OACC = [pp.tile([P, 4, 512], f32, tag=f"oacc{b}", name=f"oacc{b}")
                for b in range(B)]
        OPR = [pp.tile([P, 512], bf, tag=f"opr{jj}", name=f"opr{jj}")
               for jj in range(8)]
        # bf16 broadcast-ready reciprocal rows at partitions {0,32,64,96}
        DENRB = [pp.tile([P, 512], bf, tag=f"denrb{q4}", name=f"denrb{q4}")
                 for q4 in range(4)]
        drp = ctx.enter_context(tc.tile_pool(name="denr", bufs=2))
        vbp = ctx.enter_context(tc.tile_pool(name="vbig", bufs=4))
        ybp = ctx.enter_context(tc.tile_pool(name="ybig", bufs=2))
        ptb = ctx.enter_context(tc.tile_pool(name="ptbig", bufs=16))
        swp = ctx.enter_context(tc.tile_pool(name="swap", bufs=2))
        COS = pp.tile([P, S], bf, tag="cos")
        SIN = pp.tile([P, S], bf, tag="sin")

        # ---- PE warmup (HAM) + ACT exp-table preload, before any DMA dep ----
        nc.gpsimd.memset(WARM[:], 0.0)
        nc.gpsimd.memset(ONES[:], 1.0)
        for q4 in range(4):
            nc.gpsimd.memset(DENRB[q4][:], 0.0)
        scratch = pp.tile([1, 8], bf, tag="scratch")
        nc.scalar.activation(scratch[:], WARM[0:1, 0:8], EXP, scale=1.0)

        pts = {}
        vba = {}
        scps = None  # created after the QKV block (PSUM budget)

        def transposes(b):
            VBA = vbp.tile([P, 16, 64], bf, tag="vb", name=f"vba{b}")
            VBB = vbp.tile([P, 16, 64], bf, tag="vb", name=f"vbb{b}")
            nc.sync.dma_start_transpose(VBA[:], VT[0:64, b * S:(b + 1) * S])
            nc.sync.dma_start_transpose(VBB[:], VT[64:128, b * S:(b + 1) * S])
            vba[b] = (VBA, VBB)

        rope_sw = {}

        def rope_swaps(b):
            """Issue the partition-block swap DMAs early, spread across
            engine DMA queues: each queue is depth-1 paced (~43 GB/s), so
            putting all 8x128KB on one ring serializes to ~20us."""
            bsl = ts(b, S)
            sws = []
            qs = [[nc.gpsimd, nc.sync, nc.gpsimd, nc.sync],
                  [nc.scalar, nc.gpsimd, nc.sync, nc.scalar]]
            for ai, A in enumerate((QA, KA)):
                SW = swp.tile([P, S], bf, tag="sw", name="sw")
                for blkk in range(4):  # partition-block swap 0<->1, 2<->3
                    src = blkk ^ 1
                    qs[ai][blkk].dma_start(SW[32 * blkk:32 * blkk + 32, :],
                                           A[32 * src:32 * src + 32, bsl])
                sws.append(SW)
            rope_sw[b] = sws

        def rope_muls(b):
            for A, SW in zip((QA, KA), rope_sw[b]):
                for ch in range(2):   # 1024-wide chunks over this batch
                    asl = slice(b * S + 1024 * ch,
                                b * S + 1024 * ch + 1024)
                    csl = ts(ch, 1024)
                    nc.vector.tensor_mul(A[:, asl], A[:, asl], COS[:, csl])
                    nc.vector.tensor_mul(SW[:, csl], SW[:, csl],
                                         SIN[:, csl])
                    nc.vector.tensor_add(A[:, asl], A[:, asl],
                                         SW[:, csl])

        def rope_cswaps(b, ch):
            """Chunked (1024-col) swap issue, multi-queue (b0 streaming)."""
            asl = slice(b * S + 1024 * ch, b * S + 1024 * ch + 1024)
            qs = [[nc.gpsimd, nc.sync, nc.gpsimd, nc.sync],
                  [nc.scalar, nc.gpsimd, nc.sync, nc.scalar]]
            sws = []
            for ai, A in enumerate((QA, KA)):
                SW = swp.tile([P, 1024], bf, tag="swc", name="swc")
                for blkk in range(4):
                    src = blkk ^ 1
                    qs[ai][blkk].dma_start(SW[32 * blkk:32 * blkk + 32, :],
                                           A[32 * src:32 * src + 32, asl])
                sws.append(SW)
            rope_sw[(b, ch)] = sws

        def rope_cmuls(b, ch):
            csl = ts(ch, 1024)
            asl = slice(b * S + 1024 * ch, b * S + 1024 * ch + 1024)
            for A, SW in zip((QA, KA), rope_sw[(b, ch)]):
                nc.vector.tensor_mul(A[:, asl], A[:, asl], COS[:, csl])
                nc.vector.tensor_mul(SW[:], SW[:], SIN[:, csl])
                nc.vector.tensor_add(A[:, asl], A[:, asl], SW[:])

        def scores_slab(b, ilist, pool, width, psp, off_range=None):
            """Row-tiled (h0/h1 interleaved) score matmuls + exp for i in
            ilist. pt columns are global-q minus qs_i. off_range optionally
            restricts the q-window (chunk-split emission)."""
            for i in ilist:
                qs_i = 512 * (i // 4)
                qext = S - qs_i
                blk = b * S + 128 * i
                if (b, i, 0) not in pts:
                    for h in range(2):
                        pts[(b, i, h)] = pool.tile([P, width], bf, tag="pt",
                                                   name=f"pt{b}_{i}_{h}")
                lo, hi = (0, qext) if off_range is None else \
                    (off_range[0], min(off_range[1], qext))
                off = lo
                while off < hi:
                    w = min(1024, hi - off)
                    ps2 = [psp.tile([P, 1024], f32, tag="sc", name=f"sc{h}")
                           for h in range(2)]
                    vf = max(0, 128 * i - (qs_i + off))
                    for qc in range(0, w, 512):
                        sub = min(512, w - qc)
                        q0 = qs_i + off + qc
                        if q0 + sub <= 128 * i:
                            continue  # fully masked chunk
                        for h in range(2):
                            hsl = slice(64 * h, 64 * h + 64)
                            nc.tensor.matmul(
                                ps2[h][:, qc:qc + sub],
                                lhsT=KA[hsl, blk:blk + 128],
                                rhs=QA[hsl, b * S + q0:b * S + q0 + sub],
                                start=True, stop=True)
                    for h in range(2):
                        pt = pts[(b, i, h)]
                        if vf < w:
                            nc.scalar.activation(pt[:, off + vf:off + w],
                                                 ps2[h][:, vf:w], EXP,
                                                 scale=0.125)
                        if vf > 0:
                            nc.gpsimd.memset(pt[:, off:off + vf], 0.0)
                    off += w
                dc = 128 * i - qs_i
                if lo <= dc < hi:  # diagonal mask only in its owning chunk
                    for h in range(2):
                        nc.gpsimd.tensor_mul(pts[(b, i, h)][:, dc:dc + 128],
                                             pts[(b, i, h)][:, dc:dc + 128],
                                             TRI[:])

        # ---- QKV projection, j-outer, interleaved with b0/g0 scores ----
        with tc.tile_pool(name="xtp", bufs=3) as xtp, \
             tc.tile_pool(name="wp", bufs=1) as wp, \
             tc.tile_pool(name="scpsb", bufs=3, space="PSUM") as scps_big, \
             tc.tile_pool(name="qkvps", bufs=2, space="PSUM") as qps:
            WQ = wp.tile([P, KT, P], bf, tag="wq")
            WK = wp.tile([P, KT, P], bf, tag="wk")
            WV = wp.tile([P, KT, P], bf, tag="wv")
            # issue order = DMA priority: first-needed tensors first
            nc.sync.dma_start(WQ[:], wq_d)
            nc.sync.dma_start(WK[:], wk_d)
            nc.sync.dma_start(WV[:], wv_d)

            # HAM warmup: fine-grained garbage matmuls while wq/xt0 stream in
            for wi in range(56):
                pw = qps.tile([P, 512], f32, tag="qkv", name="pw")
                nc.tensor.matmul(pw[:, 0:128], lhsT=WARM[:, 0:128],
                                 rhs=WARM[:, 0:128], start=True, stop=True)

            xts = {}

            def fetch(j, split=1):
                t = xtp.tile([P, KT, 512], bf, tag="xt", name=f"xt{j}")
                for s in range(split):
                    k0, k1 = KT * s // split, KT * (s + 1) // split
                    nc.sync.dma_start(t[:, k0:k1], xt_d[:, j, k0:k1])
                xts[j] = t

            fetch(0, split=4)   # j0 kt-split: first MMs start at 1/4 arrival
            fetch(1)
            nc.sync.dma_start(COS[:], cos_d)
            nc.sync.dma_start(SIN[:], sin_d)
            nc.sync.dma_start(WO[:], wo_d)
            nc.sync.dma_start(TRI[:], tri_d)
            nc.sync.dma_start(SEL2[:], sel_d)
            ncpy = 0
            for j in range(8):
                if j + 2 < 8:
                    fetch(j + 2)
                # slab pieces interleave BETWEEN proj groups so an
                # exp-backlogged slab matmul never head-of-line blocks a
                # whole QKV group (one-j delay for rope(0) still applies)
                pieces = []
                if j >= 5:
                    i0 = 2 * (j - 5)
                    pieces = [[i0], [i0 + 1]]
                for np_, (W, DST) in enumerate(((WQ, QA), (WK, KA),
                                               (WV, VT))):
                    ps = qps.tile([P, 512], f32, tag="qkv", name="ps")
                    for kt in range(KT):
                        nc.tensor.matmul(ps[:], lhsT=W[:, kt, :],
                                         rhs=xts[j][:, kt, :],
                                         start=(kt == 0), stop=(kt == KT - 1))
                    # early j: scalar (exp hasn't started, DVE owns rope);
                    # late j: vector
                    if j < 6:
                        nc.scalar.copy(DST[:, ts(j, 512)], ps[:])
                    else:
                        nc.vector.tensor_copy(DST[:, ts(j, 512)], ps[:])
                    ncpy += 1
                    if np_ >= 1 and pieces:
                        scores_slab(0, pieces.pop(0), ptb, 2048, scps_big)
                del xts[j]
                # b0 rope streams chunked: swaps at j=1/3 (right after the
                # last needed scalar copy), muls at j=2/4 while DVE is idle
                if j == 1:
                    rope_cswaps(0, 0)
                elif j == 2:
                    rope_cmuls(0, 0)
                elif j == 3:
                    rope_cswaps(0, 1)
                    transposes(0)
                elif j == 4:
                    rope_cmuls(0, 1)
                if j == 7:       # b1 rope swaps go out as soon as QA/KA done
                    rope_swaps(1)
                if j == 7:   # finish b0/g0 in-block: ACT stays fed across
                    scores_slab(0, [6, 7], ptb, 2048, scps_big)

        scps = ctx.enter_context(tc.tile_pool(name="scps", bufs=2,
                                              space="PSUM"))

        # ---- attention consumers + remaining scores groups ----
        pts_small = ctx.enter_context(tc.tile_pool(name="ptsm", bufs=16))
        pay = ctx.enter_context(tc.tile_pool(name="pay", bufs=2, space="PSUM"))
        dnp = ctx.enter_context(tc.tile_pool(name="dnp", bufs=2, space="PSUM"))

        den_banks = {}

        def ilist_for(g, j):
            return [i for i in range(8 * g, 8 * g + 8) if i <= 4 * j + 3]

        def av_den(b, g, j):
            """Col-tiled AV accumulation for query block j plus 4-way-tiled
            denominator matmuls, software-pipelined so den's same-bank WAW
            stall hides under the next i's AV pair; PSUM -> OACC copy/add."""
            ilist = ilist_for(g, j)
            if not ilist:
                return
            VBA, VBB = vba[b]
            pa = pay.tile([P, 512], f32, tag="pay", name="pa")
            half = j // 2
            key = (b, half)
            if key not in den_banks:
                den_banks[key] = dnp.tile([P, 512], f32, tag="den",
                                          name=f"den{b}_{half}")
                # keep unwritten rows finite so the full-tile reciprocal
                # in div_oproj can't produce inf/NaN for the SEL matmul
                nc.vector.memset(den_banks[key][:], 1.0)
            dbank = den_banks[key]
            glast = 0 if j <= 1 else 1

            def av_pair(n, i):
                qs_i = 512 * (i // 4)
                o0 = 512 * j - qs_i
                # leading cols with q < 128i are causal-masked zeros in pt:
                # skip them (identical accumulation, shorter stream)
                vfj = max(0, 128 * i - 512 * j)
                for h, VB in ((0, VBA), (1, VBB)):
                    nc.tensor.matmul(
                        pa[64 * h:64 * h + 64, vfj:512],
                        lhsT=VB[:, i, :],
                        rhs=pts[(b, i, h)][:, o0 + vfj:o0 + 512],
                        start=(n == 0), stop=(n == len(ilist) - 1),
                        skip_group_check=True)

            def den_pair(n, i):
                qs_i = 512 * (i // 4)
                o0 = 512 * j - qs_i
                vfj = max(0, 128 * i - 512 * j)
                for h in range(2):
                    r = 32 * (2 * (j % 2) + h)
                    nc.tensor.matmul(
                        dbank[r:r + 1, vfj:512],
                        lhsT=ONES[:, 0:1],
                        rhs=pts[(b, i, h)][:, o0 + vfj:o0 + 512],
                        start=(g == 0 and n == 0),
                        stop=(g == glast and n == len(ilist) - 1),
                        skip_group_check=True,
                        tile_position=(0, r))

            for n, i in enumerate(ilist):
                av_pair(n, i)
                if n >= 1:
                    den_pair(n - 1, ilist[n - 1])
            den_pair(len(ilist) - 1, ilist[-1])
            if g == 0:
                nc.vector.tensor_copy(OACC[b][:, j, :], pa[:])
            else:
                nc.vector.tensor_add(OACC[b][:, j, :], OACC[b][:, j, :],
                                     pa[:])

        def div_oproj(b, half, borrow=False):
            """Reciprocal of denominators, SEL broadcast, OPR, o_proj,
            store. borrow=True: park o_proj pairs in the (then-idle)
            scores PSUM pool for 3x-deep pipelining and 2-et copies --
            only safe once no scps generation is still pending."""
            q4 = b * 2 + half
            dbank = den_banks.pop((b, half))
            DENR = drp.tile([P, 512], f32, tag="denr", name="denr")
            nc.vector.reciprocal_approx_fast(DENR[:], dbank[:])
            nc.vector.tensor_copy(DENRB[q4][:], DENR[:])
            for j in (2 * half, 2 * half + 1):
                jj = b * 4 + j
                pb = pay.tile([P, 512], f32, tag="pay", name="pb")
                nc.tensor.matmul(pb[:], lhsT=SEL2[:, j % 2, :],
                                 rhs=DENRB[q4][:], start=True, stop=True)
                nc.vector.tensor_mul(OPR[jj][:], OACC[b][:, j, :], pb[:])
                yb = ybp.tile([P, 8, 512], bf, tag="yb", name="yb")
                py2 = None
                for et in range(8):
                    if borrow:
                        if et % 2 == 0:
                            py2 = scps.tile([P, 1024], f32, tag="sc",
                                            name="pyw")
                        py = py2[:, ts(et % 2, 512)]
                    else:
                        py = pay.tile([P, 512], f32, tag="pay",
                                      name="py")[:]
                    nc.tensor.matmul(py, lhsT=WO[:, ts(et, P)],
                                     rhs=OPR[jj][:], start=True, stop=True)
                    if borrow:
                        if et % 2 == 1:
                            if et % 4 == 1:
                                nc.vector.tensor_copy(
                                    yb[:, et - 1:et + 1, :], py2[:])
                            else:
                                nc.scalar.copy(
                                    yb[:, et - 1:et + 1, :], py2[:])
                            nc.sync.dma_start(
                                yt_d[jj][:, et - 1:et + 1, :],
                                yb[:, et - 1:et + 1, :])
                        continue
                    # b1 runs at the tail when ACT is exp-free: go 1:1 v/s
                    if (et % 2 == 1) if b == 1 else (et % 3 == 2):
                        nc.scalar.copy(yb[:, et, :], py)
                    else:
                        nc.vector.tensor_copy(yb[:, et, :], py)
                    if et % 2 == 1:  # stream out per 2-et chunk
                        nc.sync.dma_start(yt_d[jj][:, et - 1:et + 1, :],
                                          yb[:, et - 1:et + 1, :])

        # ---- hand-scheduled interleave ----
        # Consumers first at the QKV transition (no exp dependency, keeps the
        # PE hot while DVE does rope(1)/div chains); b1's wide slabs start as
        # early as rope(1) allows so ACT never starves.
        def sl(b, i0, pool, w):
            scores_slab(b, [i0, i0 + 1], pool, w, scps)

        transposes(1)
        av_den(0, 0, 0)
        sl(0, 8, pts_small, 1024)
        av_den(0, 0, 1)
        av_den(0, 0, 2)
        rope_muls(1)
        sl(0, 10, pts_small, 1024)
        av_den(0, 0, 3)
        div_oproj(0, 0)
        sl(0, 12, pts_small, 1024)
        sl(0, 14, pts_small, 1024)
        av_den(0, 1, 2)
        sl(1, 0, ptb, 2048)
        av_den(0, 1, 3)
        sl(1, 2, ptb, 2048)
        div_oproj(0, 1)
        sl(1, 4, ptb, 2048)
        av_den(1, 0, 0)
        sl(1, 6, ptb, 2048)
        av_den(1, 0, 1)
        av_den(1, 0, 2)
        sl(1, 8, pts_small, 1024)
        av_den(1, 0, 3)
        div_oproj(1, 0)
        sl(1, 10, pts_small, 1024)
        sl(1, 12, pts_small, 1024)
        av_den(1, 1, 2)
        sl(1, 14, pts_small, 1024)
        av_den(1, 1, 3)
        div_oproj(1, 1, borrow=True)

    nc.compile()
    return nc


def get_nc():
    global _CACHED_NC
    if _CACHED_NC is None:
        _CACHED_NC = _build_nc()
    return _CACHED_NC


def run_on_hw(in_maps, **kwargs):
    from concourse.bass_utils import run_bass_kernel_spmd
    nc = get_nc()
    return run_bass_kernel_spmd(nc, in_maps, core_ids=list(range(NCORES)),
                                **kwargs)


def gather(results):
    acc = np.zeros((D, BS), np.float32)
    for r in results:
        yt = np.asarray(r["yt"]).astype(np.float32)    # [8, 128, 8, 512]
        acc += yt.transpose(2, 1, 0, 3).reshape(D, BS)
    return np.ascontiguousarray(acc.T).reshape(B, S, D).astype(np.float32)


def kernel(x, token_positions, W_qkv, W_o):
    in_maps = _host_prep(x, token_positions, W_qkv, W_o)
    res = run_on_hw(in_maps)
    return gather(res.results)

